# revision 6
# baseline (speedup 1.0000x reference)
"""Trainium2 Bass kernel for the Dale CB-cell step.

Math (per batch column b, H=48, IN=8):
    v      = hidden[b, :]                    (carried state)
    r      = sigmoid(v)
    zpre   = Ksp @ r + P_z @ x[:, b] + b_z
    u      = DT*(W @ r + P_masked @ x[:, b] + b_v)
    v_new  = v * (1 - DT*sigmoid(zpre)) + u

All 48-dim weights are folded on the host into one block-diagonal
(128, 192) bf16 matrix `rhsw` so a single PE matmul per 128-batch
subtile-pair produces [zpre | u] for both subtiles, including biases
(via a constant ones-row in the activations).

Device data layout (per core, pure batch data-parallel over 8 cores):
  - macro-tile = 2048 batch rows, held as SBUF (128p, 16*48) with
    batch = t0 + 16*p + c  (p = partition, c = subtile) so each
    partition's DMA run is 16 contiguous DRAM rows (3 KB descriptors).
  - r = sigmoid(hidden) is written bf16 into 64-col padded blocks and
    moved to H-major layout with one DMA-xbar transpose per macro-tile;
    x/ones/zero rows (host-prepacked, pre-permuted to this layout) are
    DMA'd into the 16 pad partitions of each 128-col chunk.
  - one matmul per chunk: lhsT = (128, 128) activations, rhs = const
    weights, PSUM gets (128p=batch, 192) = [z|u]x{even,odd} per group.
  - epilogue (all batch-major, full 128 partitions): ACT sigmoid on the
    z columns, DVE g = 1 - DT*z, DVE v*g, DVE + u, store.
"""

import sys

if "/opt/trn_rl_repo" not in sys.path:
    sys.path.insert(0, "/opt/trn_rl_repo")

import numpy as np

H = 48
IN = 8
DT = 0.1
B = 1048576
N_CORES = 8
B_CORE = B // N_CORES          # 131072
MACRO = 2048                   # batch rows per macro-tile
N_SUB = MACRO // 128           # 16 subtiles per macro
N_CHUNK = N_SUB // 2           # 8 matmul chunks (2 subtiles each)
GQ = 256                       # psum column stride per chunk (bank-safe)

_NC_CACHE = {}


def _softplus64(x):
    x = x.astype(np.float64)
    return np.log1p(np.exp(-np.abs(x))) + np.maximum(x, 0.0)


def _build_rhs(P, b_v, K, C, P_z, b_z, e_e, e_i):
    """Host fold of all weights into the (128, 192) matmul rhs."""
    Ksp = _softplus64(K)
    Csp = _softplus64(C)
    S = Ksp + Csp
    e_e = float(np.asarray(e_e).reshape(-1)[0])
    e_i = float(np.asarray(e_i).reshape(-1)[0])
    W_E = np.maximum(e_e * S[:, : H // 2], 0.0)
    W_I = -np.maximum(-(e_i * S[:, H // 2 :]), 0.0)
    W = np.concatenate([W_E, W_I], axis=1)          # (H, H)
    rows = np.arange(H)
    keep = ~(((rows >= H // 4) & (rows < H // 2)) | (rows >= 3 * H // 4))
    P_masked = P.astype(np.float64) * keep[:, None]

    blk = np.zeros((64, 96), np.float64)
    blk[0:H, 0:H] = Ksp.T                     # z-half:  Ksp @ r
    blk[0:H, H : 2 * H] = (DT * W).T          # u-half:  DT * W @ r
    blk[H : H + IN, 0:H] = P_z.astype(np.float64).T
    blk[H : H + IN, H : 2 * H] = (DT * P_masked).T
    blk[H + IN, 0:H] = b_z.astype(np.float64).reshape(-1)
    blk[H + IN, H : 2 * H] = DT * b_v.astype(np.float64).reshape(-1)
    rhs = np.zeros((128, 192), np.float64)
    rhs[0:64, 0:96] = blk                     # even subtile rows
    rhs[64:128, 96:192] = blk                 # odd subtile rows
    return rhs


def _build_xpad(x):
    """(16, B) = [x; ones; zeros] permuted to the device batch layout.

    Device reads xpad[i, t0 + 256*cc + 128*a + e] as the x row for batch
    index t0 + 16*e + 2*cc + a.
    """
    xz = np.zeros((16, x.shape[1]), np.float32)
    xz[0:IN] = x
    xz[IN] = 1.0
    v = xz.reshape(16, -1, 128, 16)           # [i, m, e, s]
    v = v.reshape(16, v.shape[1], 128, 8, 2)  # [i, m, e, cc, a]
    w = np.ascontiguousarray(v.transpose(0, 1, 3, 4, 2))  # [i, m, cc, a, e]
    return w.reshape(16, x.shape[1])


def _build_nc(b_core, reps=1):
    """reps>1 repeats the whole body in one NEFF (for delta-timing)."""
    import concourse.bacc as bacc
    import concourse.mybir as mybir
    import concourse.tile as tile

    F32 = mybir.dt.float32
    BF16 = mybir.dt.bfloat16
    SIG = mybir.ActivationFunctionType.Sigmoid

    n_macro = b_core // MACRO
    nc = bacc.Bacc("TRN2", target_bir_lowering=False, debug=False)
    hid = nc.dram_tensor("hidden", [b_core, H], F32, kind="ExternalInput")
    xpad = nc.dram_tensor("xpad", [16, b_core], BF16, kind="ExternalInput")
    rhsw = nc.dram_tensor("rhsw", [128, 192], BF16, kind="ExternalInput")
    out = nc.dram_tensor("out", [b_core, H], F32, kind="ExternalOutput")

    FW = N_SUB * H                            # 768  f32 working width
    RW = N_SUB * 64                           # 1024 bf16 padded width

    with tile.TileContext(nc) as tc:
        with (
            tc.tile_pool(name="const", bufs=1) as cpool,
            tc.tile_pool(name="io", bufs=3) as iopool,
            tc.tile_pool(name="work", bufs=2) as wpool,
            tc.tile_pool(name="psum", bufs=2, space="PSUM") as ppool,
        ):
            rhs_sb = cpool.tile([128, 192], BF16)
            nc.sync.dma_start(rhs_sb[:], rhsw[:])

            # r staging buffers are manually double-buffered so their pad
            # columns can be zeroed exactly once (pool slot rotation would
            # leave junk/NaN bits there for the DMA transpose to read).
            rbm_bufs = [
                cpool.tile([128, RW], BF16, name=f"rbm{i}", tag=f"rbm{i}")
                for i in range(2)
            ]
            for rb in rbm_bufs:
                pad = rb[:].rearrange("p (c e) -> p c e", e=64)[:, :, H:64]
                nc.gpsimd.memset(pad, 0.0)

            for m in range(n_macro * reps):
                t0 = (m % n_macro) * MACRO

                hv = iopool.tile([128, FW], F32, tag="hv")
                hv3 = hv[:].rearrange("p (c h) -> p c h", h=H)
                nc.sync.dma_start(
                    hv3, hid[t0 : t0 + MACRO, :].rearrange("(p c) h -> p c h", c=N_SUB)
                )

                # r = sigmoid(v), bf16, in 64-col padded blocks
                rbm = rbm_bufs[m % 2]
                rb3 = rbm[:].rearrange("p (c e) -> p c e", e=64)[:, :, 0:H]
                nc.scalar.activation(rb3, hv3, SIG)

                # H-major activations: chunk cc of lhsT = transpose of
                # rbm cols [128cc, 128cc+128)
                lhsT = wpool.tile([128, RW], BF16, tag="lhsT")
                lt3 = lhsT[:].rearrange("p (c e) -> p c e", e=128)
                nc.sync.dma_start(lt3, rbm[:], transpose=True)

                # x/ones/zeros into the pad partitions
                xsrc = xpad[:, t0 : t0 + MACRO].rearrange("i (c e) -> i c e", e=256)
                nc.sync.dma_start(
                    lhsT[48:64, :].rearrange("p (c e) -> p c e", e=128),
                    xsrc[:, :, 0:128],
                )
                nc.sync.dma_start(
                    lhsT[112:128, :].rearrange("p (c e) -> p c e", e=128),
                    xsrc[:, :, 128:256],
                )

                ps = ppool.tile([128, N_CHUNK * GQ], F32, tag="ps")
                for cc in range(N_CHUNK):
                    nc.tensor.matmul(
                        ps[:, GQ * cc : GQ * cc + 192],
                        lhsT[:, 128 * cc : 128 * cc + 128],
                        rhs_sb[:],
                        start=True,
                        stop=True,
                    )

                ps4 = (
                    ps[:]
                    .rearrange("p (g q) -> p g q", q=GQ)[:, :, 0:192]
                    .rearrange("p g (a x) -> p g a x", x=96)
                )
                ps_z = ps4[:, :, :, 0:H]
                ps_u = ps4[:, :, :, H : 2 * H]

                zs = wpool.tile([128, FW], F32, tag="zs")
                zs4 = zs[:].rearrange("p (g a x) -> p g a x", g=N_CHUNK, a=2)
                nc.scalar.activation(zs4, ps_z, SIG)

                gt = wpool.tile([128, FW], F32, tag="gt")
                nc.vector.tensor_scalar(
                    gt[:], zs[:], -DT, 1.0, mybir.AluOpType.mult, mybir.AluOpType.add
                )

                vt = wpool.tile([128, FW], F32, tag="vt")
                nc.vector.tensor_mul(vt[:], hv[:], gt[:])

                ot = iopool.tile([128, FW], F32, tag="ot")
                ot4 = ot[:].rearrange("p (g a x) -> p g a x", g=N_CHUNK, a=2)
                vt4 = vt[:].rearrange("p (g a x) -> p g a x", g=N_CHUNK, a=2)
                nc.vector.tensor_add(ot4, vt4, ps_u)

                nc.sync.dma_start(
                    out[t0 : t0 + MACRO, :].rearrange("(p c) h -> p c h", c=N_SUB),
                    ot[:].rearrange("p (c h) -> p c h", h=H),
                )

    nc.compile()
    return nc


def get_nc(b_core=B_CORE, reps=1):
    key = (b_core, reps)
    if key not in _NC_CACHE:
        _NC_CACHE[key] = _build_nc(b_core, reps)
    return _NC_CACHE[key]


def prepare_inputs(hidden, x, P, b_v, K, C, P_z, b_z, e_e, e_i, n_cores=N_CORES):
    """Host-side prep: returns per-core in_maps."""
    import ml_dtypes

    bf16 = ml_dtypes.bfloat16
    hidden = np.ascontiguousarray(np.asarray(hidden, np.float32))
    x = np.asarray(x, np.float32)
    rhs = _build_rhs(
        np.asarray(P), np.asarray(b_v), np.asarray(K), np.asarray(C),
        np.asarray(P_z), np.asarray(b_z), np.asarray(e_e), np.asarray(e_i),
    ).astype(bf16)
    xpad = _build_xpad(x).astype(bf16)
    b_core = hidden.shape[0] // n_cores
    in_maps = []
    for k in range(n_cores):
        s = slice(k * b_core, (k + 1) * b_core)
        in_maps.append(
            {
                "hidden": hidden[s],
                "xpad": np.ascontiguousarray(xpad[:, s]),
                "rhsw": rhs,
            }
        )
    return in_maps


def kernel(hidden, x, P, b_v, K, C, P_z, b_z, e_e, e_i):
    from concourse.bass_utils import run_bass_kernel_spmd

    nc = get_nc(B_CORE)
    in_maps = prepare_inputs(hidden, x, P, b_v, K, C, P_z, b_z, e_e, e_i)
    res = run_bass_kernel_spmd(nc, in_maps, list(range(N_CORES)))
    out = np.concatenate([r["out"] for r in res.results], axis=0)
    return out.astype(np.float32)


# revision 25
# speedup vs baseline: 7.0817x; 7.0817x over previous
"""Trainium2 Bass kernel for the Dale CB-cell step.

Math (per batch column b, H=48, IN=8):
    v      = hidden[b, :]                    (carried state)
    r      = sigmoid(v)
    zpre   = Ksp @ r + P_z @ x[:, b] + b_z
    u      = DT*(W @ r + P_masked @ x[:, b] + b_v)
    v_new  = v * (1 - DT*sigmoid(zpre)) + u

All 48-dim weights are folded on the host into one block-diagonal
(128, 192) bf16 matrix `rhsw` so a single PE matmul per 128-batch
subtile-pair produces [zpre | u] for both subtiles, including biases
(via a constant ones-row in the activations).

Device data layout (per core, pure batch data-parallel over 8 cores):
  - macro-tile = 2048 batch rows, held as SBUF (128p, 16*48) with
    batch = t0 + 16*p + c  (p = partition, c = subtile) so each
    partition's DMA run is 16 contiguous DRAM rows (3 KB descriptors).
  - r = sigmoid(hidden) is written bf16 into 64-col padded blocks and
    moved to H-major layout with one DMA-xbar transpose per macro-tile;
    x/ones/zero rows (host-prepacked, pre-permuted to this layout) are
    DMA'd into the 16 pad partitions of each 128-col chunk.
  - one matmul per chunk: lhsT = (128, 128) activations, rhs = const
    weights, PSUM gets (128p=batch, 192) = [z|u]x{even,odd} per group.
  - epilogue (all batch-major, full 128 partitions): ACT sigmoid on the
    z columns, DVE g = 1 - DT*z, DVE v*g, DVE + u, store.
"""

import sys

if "/opt/trn_rl_repo" not in sys.path:
    sys.path.insert(0, "/opt/trn_rl_repo")

import numpy as np

H = 48
IN = 8
DT = 0.1
B = 1048576
N_CORES = 8
B_CORE = B // N_CORES          # 131072
MACRO = 2048                   # batch rows per macro-tile
N_SUB = MACRO // 128           # 16 subtiles per macro
N_CHUNK = N_SUB // 2           # 8 matmul chunks (2 subtiles each)
GQ = 256                       # psum column stride per chunk (bank-safe)

_NC_CACHE = {}


def _softplus64(x):
    x = x.astype(np.float64)
    return np.log1p(np.exp(-np.abs(x))) + np.maximum(x, 0.0)


def _build_rhs(P, b_v, K, C, P_z, b_z, e_e, e_i):
    """Host fold of all weights into the (128, 192) matmul rhs."""
    Ksp = _softplus64(K)
    Csp = _softplus64(C)
    S = Ksp + Csp
    e_e = float(np.asarray(e_e).reshape(-1)[0])
    e_i = float(np.asarray(e_i).reshape(-1)[0])
    W_E = np.maximum(e_e * S[:, : H // 2], 0.0)
    W_I = -np.maximum(-(e_i * S[:, H // 2 :]), 0.0)
    W = np.concatenate([W_E, W_I], axis=1)          # (H, H)
    rows = np.arange(H)
    keep = ~(((rows >= H // 4) & (rows < H // 2)) | (rows >= 3 * H // 4))
    P_masked = P.astype(np.float64) * keep[:, None]

    blk = np.zeros((64, 96), np.float64)
    blk[0:H, 0:H] = Ksp.T                     # z-half:  Ksp @ r
    blk[0:H, H : 2 * H] = (DT * W).T          # u-half:  DT * W @ r
    blk[H : H + IN, 0:H] = P_z.astype(np.float64).T
    blk[H : H + IN, H : 2 * H] = (DT * P_masked).T
    blk[H + IN, 0:H] = b_z.astype(np.float64).reshape(-1)
    blk[H + IN, H : 2 * H] = DT * b_v.astype(np.float64).reshape(-1)
    rhs = np.zeros((128, 192), np.float64)
    rhs[0:64, 0:96] = blk                     # even subtile rows
    rhs[64:128, 96:192] = blk                 # odd subtile rows
    return rhs


def _build_xpad(x):
    """(16, B) = [x; ones; zeros] permuted to the device batch layout.

    Device reads xpad[i, t0 + 256*cc + 128*a + e] as the x row for batch
    index t0 + 16*e + 2*cc + a.
    """
    xz = np.zeros((16, x.shape[1]), np.float32)
    xz[0:IN] = x
    xz[IN] = 1.0
    v = xz.reshape(16, -1, 128, 16)           # [i, m, e, s]
    v = v.reshape(16, v.shape[1], 128, 8, 2)  # [i, m, e, cc, a]
    w = np.ascontiguousarray(v.transpose(0, 1, 3, 4, 2))  # [i, m, cc, a, e]
    return w.reshape(16, x.shape[1])


def _build_nc(b_core, reps=1, stage=5, bench=False):
    """reps>1 repeats the whole body in one NEFF (for delta-timing).

    stage: ablation ladder for bottleneck isolation (5 = full kernel):
      0 DMA only (hv load + x injects + store hv)
      1 + ACT r-sigmoid + DMA-transpose (consumed via tiny scratch store)
      2 + matmuls (psum slice consumed via tiny scratch store)
      3 + ACT z-sigmoid (zs slice consumed via tiny scratch store)
      4 + DVE g/v_term (vt stored as output)
      5 full
    """
    import concourse.bacc as bacc
    import concourse.mybir as mybir
    import concourse.tile as tile

    F32 = mybir.dt.float32
    BF16 = mybir.dt.bfloat16
    SIG = mybir.ActivationFunctionType.Sigmoid

    n_macro = b_core // MACRO
    nc = bacc.Bacc("TRN2", target_bir_lowering=False, debug=False)
    # bench mode: big tensors are device-internal (uninitialized) so runs
    # carry no host<->device transfer; timing-only, results meaningless.
    big = "Internal" if bench else None
    hid = nc.dram_tensor("hidden", [b_core, H], F32, kind=big or "ExternalInput")
    xpad = nc.dram_tensor("xpad", [16, b_core], BF16, kind=big or "ExternalInput")
    rhsw = nc.dram_tensor("rhsw", [128, 192], BF16, kind="ExternalInput")
    out = nc.dram_tensor("out", [b_core, H], F32, kind=big or "ExternalOutput")
    dbg = nc.dram_tensor("dbg", [128, 64], F32, kind="ExternalOutput") if bench else None
    scratch = (
        nc.dram_tensor("scratch", [128, 256], F32) if stage in (2, 3) else None
    )
    scratchb = (
        nc.dram_tensor("scratchb", [128, 64], BF16) if stage in (0, 1) else None
    )

    FW = N_SUB * H                            # 768  f32 working width
    RW = N_SUB * 64                           # 1024 bf16 padded width

    with tile.TileContext(nc) as tc:
        with (
            tc.tile_pool(name="const", bufs=1) as cpool,
            tc.tile_pool(name="io", bufs=3) as iopool,
            tc.tile_pool(name="work", bufs=2) as wpool,
            tc.tile_pool(name="psum", bufs=2, space="PSUM") as ppool,
        ):
            rhs_sb = cpool.tile([128, 192], BF16)
            nc.sync.dma_start(rhs_sb[:], rhsw[:])

            # r staging buffers are manually double-buffered so their pad
            # columns can be zeroed exactly once (pool slot rotation would
            # leave junk/NaN bits there for the DMA transpose to read).
            rbm_bufs = [
                cpool.tile([128, RW], BF16, name=f"rbm{i}", tag=f"rbm{i}")
                for i in range(2)
            ]
            for rb in rbm_bufs:
                pad = rb[:].rearrange("p (c e) -> p c e", e=64)[:, :, H:64]
                nc.gpsimd.memset(pad, 0.0)

            for m in range(n_macro * reps):
                t0 = (m % n_macro) * MACRO

                hv = iopool.tile([128, FW], F32, tag="hv")
                hv3 = hv[:].rearrange("p (c h) -> p c h", h=H)
                nc.sync.dma_start(
                    hv3, hid[t0 : t0 + MACRO, :].rearrange("(p c) h -> p c h", c=N_SUB)
                )

                lhsT = wpool.tile([128, RW], BF16, tag="lhsT")
                if stage >= 1:
                    # r = sigmoid(v), bf16, in 64-col padded blocks
                    rbm = rbm_bufs[m % 2]
                    rb3 = rbm[:].rearrange("p (c e) -> p c e", e=64)[:, :, 0:H]
                    nc.scalar.activation(rb3, hv3, SIG)

                    # H-major activations: chunk cc of lhsT = transpose of
                    # rbm cols [128cc, 128cc+128)
                    lt3 = lhsT[:].rearrange("p (c e) -> p c e", e=128)
                    nc.sync.dma_start(lt3, rbm[:], transpose=True)

                # x/ones/zeros into the pad partitions
                xsrc = xpad[:, t0 : t0 + MACRO].rearrange("i (c e) -> i c e", e=256)
                nc.sync.dma_start(
                    lhsT[48:64, :].rearrange("p (c e) -> p c e", e=128),
                    xsrc[:, :, 0:128],
                )
                nc.sync.dma_start(
                    lhsT[112:128, :].rearrange("p (c e) -> p c e", e=128),
                    xsrc[:, :, 128:256],
                )
                if stage <= 1:
                    nc.sync.dma_start(scratchb[:, 0:64], lhsT[:, 0:64])
                    nc.sync.dma_start(
                        out[t0 : t0 + MACRO, :].rearrange("(p c) h -> p c h", c=N_SUB),
                        hv3,
                    )
                    continue

                ps = ppool.tile([128, N_CHUNK * GQ], F32, tag="ps")
                for cc in range(N_CHUNK):
                    nc.tensor.matmul(
                        ps[:, GQ * cc : GQ * cc + 192],
                        lhsT[:, 128 * cc : 128 * cc + 128],
                        rhs_sb[:],
                        start=True,
                        stop=True,
                    )
                if stage == 2:
                    tmp = wpool.tile([128, 64], F32, tag="pscopy")
                    nc.scalar.activation(
                        tmp[:], ps[:, 0:64], mybir.ActivationFunctionType.Copy
                    )
                    nc.sync.dma_start(scratch[:, 0:64], tmp[:])
                    nc.sync.dma_start(
                        out[t0 : t0 + MACRO, :].rearrange("(p c) h -> p c h", c=N_SUB),
                        hv3,
                    )
                    continue

                ps4 = (
                    ps[:]
                    .rearrange("p (g q) -> p g q", q=GQ)[:, :, 0:192]
                    .rearrange("p g (a x) -> p g a x", x=96)
                )
                ps_z = ps4[:, :, :, 0:H]
                ps_u = ps4[:, :, :, H : 2 * H]

                zs = wpool.tile([128, FW], F32, tag="zs")
                zs4 = zs[:].rearrange("p (g a x) -> p g a x", g=N_CHUNK, a=2)
                nc.scalar.activation(zs4, ps_z, SIG)
                if stage == 3:
                    nc.sync.dma_start(scratch[:, 0:64], zs[:, 0:64])
                    nc.sync.dma_start(
                        out[t0 : t0 + MACRO, :].rearrange("(p c) h -> p c h", c=N_SUB),
                        hv3,
                    )
                    continue

                gt = wpool.tile([128, FW], F32, tag="gt")
                nc.vector.tensor_scalar(
                    gt[:], zs[:], -DT, 1.0, mybir.AluOpType.mult, mybir.AluOpType.add
                )

                vt = wpool.tile([128, FW], F32, tag="vt")
                nc.vector.tensor_mul(vt[:], hv[:], gt[:])
                if stage == 4:
                    nc.sync.dma_start(
                        out[t0 : t0 + MACRO, :].rearrange("(p c) h -> p c h", c=N_SUB),
                        vt[:].rearrange("p (c h) -> p c h", h=H),
                    )
                    continue

                ot = iopool.tile([128, FW], F32, tag="ot")
                ot4 = ot[:].rearrange("p (g a x) -> p g a x", g=N_CHUNK, a=2)
                vt4 = vt[:].rearrange("p (g a x) -> p g a x", g=N_CHUNK, a=2)
                nc.vector.tensor_add(ot4, vt4, ps_u)

                nc.sync.dma_start(
                    out[t0 : t0 + MACRO, :].rearrange("(p c) h -> p c h", c=N_SUB),
                    ot[:].rearrange("p (c h) -> p c h", h=H),
                )

            if bench:
                dbg_t = cpool.tile([128, 64], F32, name="dbg_t", tag="dbg_t")
                nc.gpsimd.memset(dbg_t[:], 0.0)
                nc.sync.dma_start(dbg[:], dbg_t[:])

    nc.compile()
    return nc


MACRO2 = 8192                 # v2 macro-tile rows
N_SUB2 = MACRO2 // 128        # 64 subtiles (one per 128-col transpose chunk)
N_WIN = MACRO2 // 512         # 16 matmul windows per macro


def _build_nc_v2(b_core, reps=1, bench=False, mm_n=512):
    """v2: instruction-count-minimized design.

    Per 8192-row macro: 1 hv load, 1 sigmoid, 1 DMA-transpose (1 subtile
    per 128-chunk, K rows 0:64 = [r(48); x/ones/zeros(16)]), 1 x-inject,
    16 matmuls (lhsT = constant (64, 96) weights, rhs = 512 batch cols),
    4 DVE psum evictions to bf16, 1 DMA-transpose back to batch-major,
    then a 4-op in-place f32 epilogue + 1 store.
    """
    import concourse.bacc as bacc
    import concourse.mybir as mybir
    import concourse.tile as tile

    F32 = mybir.dt.float32
    BF16 = mybir.dt.bfloat16
    SIG = mybir.ActivationFunctionType.Sigmoid

    n_macro = b_core // MACRO2
    nc = bacc.Bacc("TRN2", target_bir_lowering=False, debug=False)
    big = "Internal" if bench else None
    hid = nc.dram_tensor("hidden", [b_core, H], F32, kind=big or "ExternalInput")
    xpad = nc.dram_tensor("xpad", [16, b_core], BF16, kind=big or "ExternalInput")
    rhsw = nc.dram_tensor("rhsw", [64, 96], BF16, kind="ExternalInput")
    out = nc.dram_tensor("out", [b_core, H], F32, kind=big or "ExternalOutput")
    dbg = nc.dram_tensor("dbg", [128, 64], F32, kind="ExternalOutput") if bench else None

    FW = N_SUB2 * H               # 3072  (f32 working width per macro)
    RW = N_SUB2 * 128             # 8192  (bf16 padded width per macro)

    with tile.TileContext(nc) as tc:
        with (
            tc.tile_pool(name="const", bufs=1) as cpool,
            tc.tile_pool(name="io", bufs=2) as iopool,
            tc.tile_pool(name="work", bufs=2) as wpool,
            tc.tile_pool(name="psum", bufs=2, space="PSUM") as ppool,
        ):
            w_sb = cpool.tile([64, 96], BF16)
            nc.sync.dma_start(w_sb[:], rhsw[:])

            # manual double-buffers for the two transpose sources so their
            # never-written pad regions can be zeroed exactly once (keeps
            # CoreSim's uninit-read check green; HW wouldn't care).
            rbm_bufs = [
                cpool.tile([128, RW], BF16, name=f"rbm2_{i}", tag=f"rbm2_{i}")
                for i in range(2)
            ]
            for rb in rbm_bufs:
                pad = rb[:].rearrange("p (c e) -> p c e", e=128)[:, :, H:128]
                nc.gpsimd.memset(pad, 0.0)
            zu_bufs = [
                cpool.tile([128, RW], BF16, name=f"zu2_{i}", tag=f"zu2_{i}")
                for i in range(2)
            ]
            for zb in zu_bufs:
                nc.gpsimd.memset(zb[96:128, :], 0.0)

            for m in range(n_macro * reps):
                t0 = (m % n_macro) * MACRO2

                # batch(p, c) = t0 + 64*p + c, c in [0, 64)
                hv = iopool.tile([128, FW], F32, tag="hv", bufs=3)
                hv3 = hv[:].rearrange("p (c h) -> p c h", h=H)
                nc.sync.dma_start(
                    hv3,
                    hid[t0 : t0 + MACRO2, :].rearrange("(p c) h -> p c h", c=N_SUB2),
                )

                # r = sigmoid(v) bf16 into 128-col padded chunks
                rbm = rbm_bufs[m % 2]
                rb3 = rbm[:].rearrange("p (c e) -> p c e", e=128)[:, :, 0:H]
                nc.scalar.activation(rb3, hv3, SIG)

                # chunk c of trans = transpose of rbm cols [128c, 128c+128):
                # rows 0:48 = r (H-major), 48:64 <- x/ones/zeros, 64:128 junk
                trans = wpool.tile([128, RW], BF16, tag="trans")
                tr3 = trans[:].rearrange("p (c e) -> p c e", e=128)
                nc.sync.dma_start(tr3, rbm[:], transpose=True)
                nc.sync.dma_start(
                    trans[48:64, :].rearrange("p (c e) -> p c e", e=128),
                    xpad[:, t0 : t0 + MACRO2].rearrange("i (c e) -> i c e", e=128),
                )

                # zu: cols 128c+e <-> batch(e, c); rows [z(48) | u(48)]
                zu = zu_bufs[m % 2]
                mm_per_ps = 2048 // mm_n
                for g in range(4):
                    ps = ppool.tile([96, 2048], F32, tag="ps")
                    for s in range(mm_per_ps):
                        w = mm_per_ps * g + s
                        nc.tensor.matmul(
                            ps[:, mm_n * s : mm_n * s + mm_n],
                            w_sb[:],
                            trans[0:64, mm_n * w : mm_n * w + mm_n],
                            start=True,
                            stop=True,
                        )
                    nc.vector.tensor_copy(
                        zu[0:96, 2048 * g : 2048 * g + 2048], ps[:]
                    )

                # back to batch-major: zuT chunk c = [z|u|junk] for batch(p, c)
                zuT = wpool.tile([128, RW], BF16, tag="zuT", bufs=3)
                zt3 = zuT[:].rearrange("p (c e) -> p c e", e=128)
                nc.sync.dma_start(zt3, zu[:], transpose=True)
                zuT4 = zuT[:].rearrange("p (c e) -> p c e", e=128)
                z_v = zuT4[:, :, 0:H]
                u_v = zuT4[:, :, H : 2 * H]

                # epilogue, all into one f32 tile:
                # acc = sigmoid(z); acc = 1 - DT*acc; acc = hv*acc; acc += u
                acc = wpool.tile([128, FW], F32, tag="acc")
                acc3 = acc[:].rearrange("p (c h) -> p c h", h=H)
                nc.scalar.activation(acc3, z_v, SIG)
                nc.vector.tensor_scalar(
                    acc[:], acc[:], -DT, 1.0, mybir.AluOpType.mult,
                    mybir.AluOpType.add,
                )
                nc.vector.tensor_mul(acc[:], hv[:], acc[:])
                nc.vector.tensor_tensor(
                    acc3, acc3, u_v, op=mybir.AluOpType.add
                )
                nc.sync.dma_start(
                    out[t0 : t0 + MACRO2, :].rearrange("(p c) h -> p c h", c=N_SUB2),
                    acc3,
                )

            if bench:
                dbg_t = cpool.tile([128, 64], F32, name="dbg_t2", tag="dbg_t2")
                nc.gpsimd.memset(dbg_t[:], 0.0)
                nc.sync.dma_start(dbg[:], dbg_t[:])

    nc.compile()
    return nc


def _build_xpad_v2(x):
    """(16, B) = [x; ones; zeros] permuted for v2's inject.

    Device reads xpad[i, t0 + 128*c + e] as the x row for batch
    t0 + 64*e + c (t0 = multiple of 8192).
    """
    xz = np.zeros((16, x.shape[1]), np.float32)
    xz[0:IN] = x
    xz[IN] = 1.0
    v = xz.reshape(16, -1, 128, 64)                       # [i, m, e, c]
    w = np.ascontiguousarray(v.transpose(0, 1, 3, 2))     # [i, m, c, e]
    return w.reshape(16, x.shape[1])


def prepare_inputs_v2(hidden, x, P, b_v, K, C, P_z, b_z, e_e, e_i, n_cores=N_CORES):
    import ml_dtypes

    bf16 = ml_dtypes.bfloat16
    hidden = np.ascontiguousarray(np.asarray(hidden, np.float32))
    x = np.asarray(x, np.float32)
    rhs128 = _build_rhs(
        np.asarray(P), np.asarray(b_v), np.asarray(K), np.asarray(C),
        np.asarray(P_z), np.asarray(b_z), np.asarray(e_e), np.asarray(e_i),
    )
    blk = rhs128[0:64, 0:96].astype(bf16)                 # (64, 96)
    xpad = _build_xpad_v2(x).astype(bf16)
    b_core = hidden.shape[0] // n_cores
    in_maps = []
    for k in range(n_cores):
        s = slice(k * b_core, (k + 1) * b_core)
        in_maps.append(
            {
                "hidden": hidden[s],
                "xpad": np.ascontiguousarray(xpad[:, s]),
                "rhsw": blk,
            }
        )
    return in_maps


def get_nc(b_core=B_CORE, reps=1, stage=5):
    key = (b_core, reps, stage)
    if key not in _NC_CACHE:
        _NC_CACHE[key] = _build_nc(b_core, reps, stage)
    return _NC_CACHE[key]


def prepare_inputs(hidden, x, P, b_v, K, C, P_z, b_z, e_e, e_i, n_cores=N_CORES):
    """Host-side prep: returns per-core in_maps."""
    import ml_dtypes

    bf16 = ml_dtypes.bfloat16
    hidden = np.ascontiguousarray(np.asarray(hidden, np.float32))
    x = np.asarray(x, np.float32)
    rhs = _build_rhs(
        np.asarray(P), np.asarray(b_v), np.asarray(K), np.asarray(C),
        np.asarray(P_z), np.asarray(b_z), np.asarray(e_e), np.asarray(e_i),
    ).astype(bf16)
    xpad = _build_xpad(x).astype(bf16)
    b_core = hidden.shape[0] // n_cores
    in_maps = []
    for k in range(n_cores):
        s = slice(k * b_core, (k + 1) * b_core)
        in_maps.append(
            {
                "hidden": hidden[s],
                "xpad": np.ascontiguousarray(xpad[:, s]),
                "rhsw": rhs,
            }
        )
    return in_maps


def get_nc_v2(b_core=B_CORE, reps=1, bench=False):
    key = ("v2", b_core, reps, bench)
    if key not in _NC_CACHE:
        _NC_CACHE[key] = _build_nc_v2(b_core, reps, bench)
    return _NC_CACHE[key]


def kernel(hidden, x, P, b_v, K, C, P_z, b_z, e_e, e_i):
    from concourse.bass_utils import run_bass_kernel_spmd

    nc = get_nc_v2(B_CORE)
    in_maps = prepare_inputs_v2(hidden, x, P, b_v, K, C, P_z, b_z, e_e, e_i)
    res = run_bass_kernel_spmd(nc, in_maps, list(range(N_CORES)))
    out = np.concatenate([r["out"] for r in res.results], axis=0)
    return out.astype(np.float32)
